# revision 62
# baseline (speedup 1.0000x reference)
"""Multi-head attention (B=4, S=2048, D=1024, H=16, causal) on 8 TRN2 NeuronCores.

Sharding: core c -> batch b = c // 2, head-group g = c % 2 (8 heads, 512 dims).
Each core computes its heads' projections + full SxS causal attention + its
partial output projection; the host sums the two head-group partials per batch
and adds the output bias.

Host-side prep: x and all weights are cast to bf16 and pre-transposed into the
layouts the PE consumes directly (x^T [D,S], W^T [D,DH], Wo^T [DH,D]), so the
kernel does zero on-device transposes and the input DMA bytes are halved.

Per-core pipeline (matmuls bf16 with fp32 PSUM accumulate; attn@V in fp8):
  - Q^T/K^T per head-pair slabs [128, S]; V kept natural with a ones column
    per head (ones-augmented V makes attn@V also produce sumexp rows)
  - scores^T tiles [128 j, 512 i] via two K=64 row-packed matmuls (both heads
    of a pair concurrently in the PE array); diagonal tiles only computed
    from their first in-range column
  - exp on ScalarE straight out of PSUM (no max subtraction: |s/8| <= 9.47
    verified empirically); fp8 path biases by -6*ln2 so exp fits fp8e4's
    +-240 range (the softmax normalization cancels the factor)
  - attn@V for i-chunks 1..3 in fp8e4 with MatmulPerfMode.DoubleRow: two
    128-row j-tiles contracted per pass (2x PE throughput). i-chunk 0 (rows
    0-511) stays bf16: its rows average few keys, so fp8 noise would not be
    sqrt(n)-attenuated there.
  - softmax normalization: 1/sumexp via one reciprocal_approx_fast, then
    gpsimd partition_broadcast, then one DVE multiply per head
  - a credit-based scheduler interleaves V/QK-projection and output-
    projection chunks between attention pair-steps, so ScalarE's exp stream
    and the PE matmul stream stay dense simultaneously
  - y emitted bf16 (host accumulates the two partials per batch in fp32)
"""

import os
import numpy as np

B, S, D = 4, 2048, 1024
H, DK = 16, 64
N_CORES = 8
DH = 512          # head dims per core (8 heads x 64)
P = 128           # partitions
KT = D // P       # 8 k-slabs
NPAIR = 4         # head pairs per core
SC = S // 512     # 4 s-chunks of 512
ST = S // P       # 16 s-tiles of 128
HW = DK + 2       # 66: per-head row width in fp8 V storage (64 V + 1 ones + 1 pad)
VWP = 8 * HW      # 528: fp8 v storage row width per s-tile (16-aligned for DoubleRow)
VW16 = 8 * (DK + 1)  # 520: bf16 v storage row width (s-tiles 0..3 only)
B_EXP = -6 * 0.6931471805599453  # exp bias: keeps exp(s/8)*2^-6 <= ~202 < 240 (fp8e4 max)

_CACHE = {}
LAST_EXEC_NS = None


def _build():
    from contextlib import ExitStack

    import concourse.bass as bass
    import concourse.tile as tile
    from concourse import bacc, mybir

    f32 = mybir.dt.float32
    bf16 = mybir.dt.bfloat16
    fp8 = mybir.dt.float8e4
    AF = mybir.ActivationFunctionType
    OP = mybir.AluOpType
    DR = mybir.MatmulPerfMode.DoubleRow

    nc = bacc.Bacc("TRN2", target_bir_lowering=False, debug=False,
                   num_devices=N_CORES)

    # host-pretransposed bf16 inputs
    xt = nc.dram_tensor("xt", [D, S], bf16, kind="ExternalInput").ap()
    wq = nc.dram_tensor("wq", [D, DH], bf16, kind="ExternalInput").ap()
    wk = nc.dram_tensor("wk", [D, DH], bf16, kind="ExternalInput").ap()
    wv = nc.dram_tensor("wv", [D, DH], bf16, kind="ExternalInput").ap()
    wo = nc.dram_tensor("wo", [DH, D], bf16, kind="ExternalInput").ap()
    bq = nc.dram_tensor("bq", [DH], f32, kind="ExternalInput").ap()
    bk = nc.dram_tensor("bk", [DH], f32, kind="ExternalInput").ap()
    bv = nc.dram_tensor("bv", [DH], f32, kind="ExternalInput").ap()
    y = nc.dram_tensor("y", [S, D], bf16, kind="ExternalOutput").ap()

    with tile.TileContext(nc) as tc, ExitStack() as ctx:
        persist = ctx.enter_context(tc.tile_pool(name="persist", bufs=1))

        # persistent SBUF tensors
        xT = persist.tile([P, KT * S], bf16, tag="xT")            # k-slab k at cols [k*S, (k+1)*S)
        x8 = persist.tile([P, KT * S], fp8, tag="x8")             # fp8 copy for V projection
        wqT = persist.tile([P, KT * DH], bf16, tag="wqT")         # [128k, 512dq] per slab
        wkT = persist.tile([P, KT * DH], bf16, tag="wkT")
        wvT = persist.tile([P, KT * DH], bf16, tag="wvT")
        wv8 = persist.tile([P, KT * DH], fp8, tag="wv8")
        woT = persist.tile([P, NPAIR * D], bf16, tag="woT")       # d-slab dt at cols [dt*D, ...)
        qT = persist.tile([P, NPAIR * S], bf16, tag="qT")         # pair p at cols [p*S, ...)
        kTt = persist.tile([P, NPAIR * S], bf16, tag="kTt")
        vS = persist.tile([P, ST * VWP], fp8, tag="vS")           # s-tile jt at cols [jt*VWP, ...)
        vS16 = persist.tile([P, 4 * VW16], bf16, tag="vS16")      # bf16 V for s-tiles 0..3
        ctxT = persist.tile([P, NPAIR * S], bf16, tag="ctxT")
        # the causal boundary strip (128x128 lower-triangle; same for every
        # diagonal offset)
        mask2 = persist.tile([P, 128], bf16, tag="mask2")
        mask8 = persist.tile([P, 128], fp8, tag="mask8")
        bq_sb = persist.tile([P, NPAIR], f32, tag="bq_sb")
        bk_sb = persist.tile([P, NPAIR], f32, tag="bk_sb")
        bv_sb = persist.tile([1, DH], f32, tag="bv_sb")
        bv_bc = persist.tile([P, DH], f32, tag="bv_bc")
        ones_f = persist.tile([P, 128], f32, tag="ones_f")
        bexp_sb = persist.tile([P, 1], f32, tag="bexp_sb")

        # ---- input DMAs spread across the issue queues ----
        # wv and x-sc0 gate the first PE work: split their descriptor
        # issues across queues so both arrive ~8us in
        def _dma_x(eng, sc, k0=0, k1=KT):
            for k in range(k0, k1):
                eng.dma_start(
                    out=xT[:, k * S + sc * 512: k * S + (sc + 1) * 512],
                    in_=xt[k * P:(k + 1) * P, sc * 512:(sc + 1) * 512])
        for k in range(KT):
            nc.scalar.dma_start(out=wvT[:, k * DH:(k + 1) * DH],
                                in_=wv[k * P:(k + 1) * P, :])
        _dma_x(nc.sync, 0, 0, 4)          # sc0 k4..7 issued by gpsimd below
        # wq/wk next on sync: the first qk chunks gate the exp ramp-up
        for k in range(KT):
            nc.sync.dma_start(out=wqT[:, k * DH:(k + 1) * DH],
                              in_=wq[k * P:(k + 1) * P, :])
        for k in range(KT):
            nc.sync.dma_start(out=wkT[:, k * DH:(k + 1) * DH],
                              in_=wk[k * P:(k + 1) * P, :])
        _dma_x(nc.scalar, 1)
        _dma_x(nc.sync, 2)
        _dma_x(nc.sync, 3)
        # gpsimd: warmup scratch first, then biases + V-ones init + bv
        # broadcast (v_proj needs them), then wq/wk/wo, then the mask/
        # constant prep (needed later)
        warm_sb = persist.tile([P, 512], bf16, tag="warm_sb")
        nc.gpsimd.memset(warm_sb[:], 0.5)
        nc.gpsimd.dma_start(out=bq_sb[:], in_=bq[:].rearrange("(c p) -> p c", p=P))
        nc.gpsimd.dma_start(out=bk_sb[:], in_=bk[:].rearrange("(c p) -> p c", p=P))
        nc.gpsimd.dma_start(out=bv_sb[0:1, :], in_=bv[:])
        # only the ones columns need initializing; V columns are written by
        # the projection
        nc.gpsimd.memset(
            vS[:].rearrange("p (t h c) -> p t h c", h=8, c=HW)[:, :, :, DK:DK + 1], 1.0)
        nc.gpsimd.memset(
            vS16[:].rearrange("p (t h c) -> p t h c", h=8, c=DK + 1)[:, :, :, DK:DK + 1], 1.0)
        nc.gpsimd.partition_broadcast(bv_bc[:], bv_sb[0:1, :])
        _dma_x(nc.gpsimd, 0, 4, KT)
        for dt in range(NPAIR):
            nc.gpsimd.dma_start(out=woT[:, dt * D:(dt + 1) * D],
                                in_=wo[dt * P:(dt + 1) * P, :])
        nc.gpsimd.memset(ones_f[:], 1.0)
        nc.gpsimd.memset(bexp_sb[:], B_EXP)
        for mk in (mask2, mask8):
            # keep iff strip-local column >= channel (lower triangle)
            nc.gpsimd.affine_select(
                out=mk[:],
                in_=ones_f[:],
                pattern=[[1, 128]],
                compare_op=OP.is_ge,
                fill=0.0,
                base=0,
                channel_multiplier=-1,
            )

        ps_small = ctx.enter_context(tc.tile_pool(name="ps_small", bufs=2, space="PSUM"))
        ps_scores = ctx.enter_context(tc.tile_pool(name="ps_scores", bufs=2, space="PSUM"))
        ps_acc = ctx.enter_context(tc.tile_pool(name="ps_acc", bufs=2, space="PSUM"))
        sb_vtmp = ctx.enter_context(tc.tile_pool(name="sb_vtmp", bufs=2))
        sb_exp8 = ctx.enter_context(tc.tile_pool(name="sb_exp8", bufs=7))
        sb_exp16 = ctx.enter_context(tc.tile_pool(name="sb_exp16", bufs=6))
        sb_y = ctx.enter_context(tc.tile_pool(name="sb_y", bufs=4))
        sb_rab = ctx.enter_context(tc.tile_pool(name="sb_rab", bufs=2))
        sb_rsb = ctx.enter_context(tc.tile_pool(name="sb_rsb", bufs=3))

        # ---- work-chunk emitters (filler units for the scheduler) ----
        def x8_chunk(sc):
            # fp8 copy of x for the DoubleRow V projection
            for k in range(KT):
                nc.vector.tensor_copy(
                    x8[:, k * S + sc * 512: k * S + (sc + 1) * 512],
                    xT[:, k * S + sc * 512: k * S + (sc + 1) * 512])

        def wv8_cast():
            # split so no single long cast blocks the DVE FIFO
            for k in range(0, KT, 2):
                nc.vector.tensor_copy(wv8[:, k * DH:(k + 2) * DH],
                                      wvT[:, k * DH:(k + 2) * DH])

        def v_chunk(st):
            vp = ps_small.tile([P, 512], f32, tag="work", name=f"vps{st}")
            if st < 4:
                for k in range(KT):
                    nc.tensor.matmul(
                        vp[:],
                        xT[:, k * S + st * P: k * S + (st + 1) * P],
                        wvT[:, k * DH:(k + 1) * DH],
                        start=(k == 0), stop=(k == KT - 1))
            else:
                # fp8 DoubleRow: two k-slabs contracted per pass
                for k2 in range(KT // 2):
                    lhs = x8[:, 2 * k2 * S: (2 * k2 + 2) * S].rearrange(
                        "p (two s) -> p two s", two=2)[:, :, st * P:(st + 1) * P]
                    rhs = wv8[:, 2 * k2 * DH: (2 * k2 + 2) * DH].rearrange(
                        "p (two d) -> p two d", two=2)
                    nc.tensor.matmul(
                        vp[:], lhs, rhs, perf_mode=DR,
                        start=(k2 == 0), stop=(k2 == KT // 2 - 1))
            vdst8 = vS[:, st * VWP:(st + 1) * VWP].rearrange(
                "p (h c) -> p h c", c=HW)[:, :, 0:DK]
            if st < 4:
                vt = sb_vtmp.tile([P, 512], f32, tag="vtmp", name=f"vt{st}")
                nc.vector.tensor_tensor(vt[:], vp[:], bv_bc[:], OP.add)
                nc.vector.tensor_copy(vdst8, vt[:].rearrange("p (h c) -> p h c", c=DK))
                vdst16 = vS16[:, st * VW16:(st + 1) * VW16].rearrange(
                    "p (h c) -> p h c", c=DK + 1)[:, :, 0:DK]
                nc.scalar.copy(vdst16, vt[:].rearrange("p (h c) -> p h c", c=DK))
            else:
                # fused bias-add + fp8 cast straight out of PSUM
                nc.vector.tensor_tensor(
                    vdst8,
                    vp[:].rearrange("p (h c) -> p h c", c=DK),
                    bv_bc[:].rearrange("p (h c) -> p h c", c=DK),
                    OP.add)

        def qk_chunk(name, p, sc):
            wT, bias_sb, out_sb = ((wqT, bq_sb, qT) if name == "q"
                                   else (wkT, bk_sb, kTt))
            pw = ps_small.tile([P, 512], f32, tag="work", name=f"{name}ps{p}_{sc}")
            for k in range(KT):
                nc.tensor.matmul(
                    pw[:],
                    wT[:, k * DH + p * P: k * DH + (p + 1) * P],
                    xT[:, k * S + sc * 512: k * S + (sc + 1) * 512],
                    start=(k == 0), stop=(k == KT - 1))
            nc.vector.tensor_scalar_add(
                out_sb[:, p * S + sc * 512: p * S + (sc + 1) * 512],
                pw[:], bias_sb[:, p:p + 1])

        def oproj_half(ic, st, mc, tail=False):
            yp = ps_small.tile([P, 512], f32, tag="work", name=f"yp{st}_{mc}")
            for dt in range(NPAIR):
                nc.tensor.matmul(
                    yp[:],
                    ctxT[:, dt * S + st * P: dt * S + (st + 1) * P],
                    woT[:, dt * D + mc * 512: dt * D + (mc + 1) * 512],
                    start=(dt == 0), stop=(dt == NPAIR - 1))
            yt = sb_y.tile([P, 512], bf16, tag="yout", name=f"yt{st}_{mc}")
            nc.vector.tensor_copy(yt[:], yp[:])
            eng = nc.gpsimd if tail else nc.sync
            eng.dma_start(
                out=y[st * P:(st + 1) * P, mc * 512:(mc + 1) * 512], in_=yt[:])

        # ---- attention group step generators ----
        def attn_drain(ic, p, accA, accB, tail=False):
            cslice = slice(p * S + ic * 512, p * S + (ic + 1) * 512)
            # sumexp rows first: the gpsimd broadcast chain is the critical
            # path, the ctx copies slot in behind it on the DVE FIFO
            sraw = sb_rab.tile([1, 1024], f32, tag="sraw", name=f"sr{ic}{p}")
            nc.vector.tensor_copy(sraw[0:1, 0:512], accA[DK:DK + 1, :])
            nc.vector.tensor_copy(sraw[0:1, 512:1024], accB[DK:DK + 1, :])
            Rsr = sb_rsb.tile([P, 1024], f32, tag="rsb", name=f"rr{ic}{p}")
            nc.gpsimd.partition_broadcast(Rsr[:], sraw[0:1, :])
            if tail:
                # ScalarE has finished its exps by the final drain
                nc.scalar.copy(ctxT[0:DK, cslice], accA[0:DK, :])
                nc.scalar.copy(ctxT[DK:P, cslice], accB[0:DK, :])
            else:
                nc.vector.tensor_copy(ctxT[0:DK, cslice], accA[0:DK, :])
                nc.vector.tensor_copy(ctxT[DK:P, cslice], accB[0:DK, :])
            # reciprocal across all 128 lanes (a [1,N] reciprocal would run
            # on a single DVE lane)
            Rs = sb_rsb.tile([P, 1024], f32, tag="rsb", name=f"rs{ic}{p}")
            nc.vector.reciprocal_approx_fast(Rs[:], Rsr[:])
            for hl in (0, 1):
                csl = ctxT[hl * DK:(hl + 1) * DK, cslice]
                nc.vector.tensor_mul(
                    csl, csl, Rs[hl * DK:(hl + 1) * DK, hl * 512:(hl + 1) * 512])

        def attn_steps_bf16(ic, p):
            # per-tile bf16 path (ic=0 only: rows 0-511)
            accA = ps_acc.tile([DK + 1, 512], f32, tag="acc", name=f"accA{ic}_{p}")
            accB = ps_acc.tile([DK + 1, 512], f32, tag="acc", name=f"accB{ic}_{p}")
            njt = 4 * ic + 4
            exs = {}
            dof = {}

            def attn_mm(hl, jt):
                acc = accA if hl == 0 else accB
                d = dof[jt]
                hv = 2 * p + hl
                nc.tensor.matmul(
                    acc[:, d:512],
                    vS16[:, jt * VW16 + hv * (DK + 1): jt * VW16 + (hv + 1) * (DK + 1)],
                    exs[jt][:, hl * 512 + d:(hl + 1) * 512],
                    start=(jt == 0), stop=(jt == njt - 1))

            for jt in range(njt):
                sps = ps_scores.tile([P, 1024], f32, tag="scores", name=f"sps{ic}{p}{jt}")
                diag = jt >= 4 * ic
                d = (jt - 4 * ic) * P if diag else 0
                dof[jt] = d
                nc.tensor.matmul(
                    sps[:, d:512],
                    kTt[0:DK, p * S + jt * P: p * S + (jt + 1) * P],
                    qT[0:DK, p * S + ic * 512 + d: p * S + (ic + 1) * 512],
                    start=True, stop=True)
                nc.tensor.matmul(
                    sps[:, 512 + d:1024],
                    kTt[DK:P, p * S + jt * P: p * S + (jt + 1) * P],
                    qT[DK:P, p * S + ic * 512 + d: p * S + (ic + 1) * 512],
                    start=True, stop=True)
                ex = sb_exp16.tile([P, 1024], bf16, tag="exp16", name=f"ex{ic}{p}{jt}")
                exs[jt] = ex
                nc.scalar.activation(ex[:, d:1024], sps[:, d:1024], AF.Exp, scale=0.125)
                if diag:
                    # only the 128-wide boundary strip is partially masked;
                    # columns beyond d+128 are fully in-range
                    for hb in (0, 512):
                        nc.vector.tensor_mul(
                            ex[:, hb + d: hb + d + 128],
                            ex[:, hb + d: hb + d + 128],
                            mask2[:])
                attn_mm(0, jt)
                if jt >= 2:
                    attn_mm(1, jt - 2)
                yield 0.15
            for jt in range(max(0, njt - 2), njt):
                attn_mm(1, jt)
            attn_drain(ic, p, accA, accB)

        def attn_steps_fp8(ic, p, hb=False):
            # fp8 DoubleRow path: j-tiles consumed in pairs, 2x PE throughput
            accA = ps_acc.tile([DK + 1, 512], f32, tag="acc", name=f"accA{ic}_{p}")
            accB = ps_acc.tile([DK + 1, 512], f32, tag="acc", name=f"accB{ic}_{p}")
            npair = 2 * ic + 2
            exs = {}
            wss = {}

            def attn_mm(hl, m):
                acc = accA if hl == 0 else accB
                ws = wss[m]
                hv = 2 * p + hl
                lhs = vS[:, 2 * m * VWP: (2 * m + 2) * VWP].rearrange(
                    "p (two w) -> p two w", two=2)[:, :, hv * HW: hv * HW + DK + 1]
                rhs = exs[m].rearrange(
                    "p (two w) -> p two w", two=2)[:, :, hl * 512 + ws: (hl + 1) * 512]
                nc.tensor.matmul(
                    acc[:, ws:512], lhs, rhs, perf_mode=DR,
                    start=(m == 0), stop=(m == npair - 1))

            for m in range(npair):
                jts = (2 * m, 2 * m + 1)
                ex2 = sb_exp8.tile([P, 2048], fp8, tag="exp8", name=f"e2{ic}{p}{m}")
                exs[m] = ex2
                d0 = (jts[0] - 4 * ic) * P if jts[0] >= 4 * ic else 0
                wss[m] = d0
                for q01, jt in enumerate(jts):
                    diag = jt >= 4 * ic
                    d = (jt - 4 * ic) * P if diag else 0
                    sps = ps_scores.tile([P, 1024], f32, tag="scores",
                                         name=f"sps{ic}{p}{jt}")
                    if hb and q01 == 0:
                        # dummy matmul: overwritten by the real start=True
                        # below; exists only to hold the PE clock at 2.4GHz
                        nc.tensor.matmul(sps[:, 0:512], warm_sb[:, 0:P],
                                         warm_sb[:], start=True, stop=True)
                    nc.tensor.matmul(
                        sps[:, d:512],
                        kTt[0:DK, p * S + jt * P: p * S + (jt + 1) * P],
                        qT[0:DK, p * S + ic * 512 + d: p * S + (ic + 1) * 512],
                        start=True, stop=True)
                    nc.tensor.matmul(
                        sps[:, 512 + d:1024],
                        kTt[DK:P, p * S + jt * P: p * S + (jt + 1) * P],
                        qT[DK:P, p * S + ic * 512 + d: p * S + (ic + 1) * 512],
                        start=True, stop=True)
                    off = q01 * 1024
                    if diag and d > d0:
                        # window [d0:512] streams this tile's A-side cols
                        # [d0:d) which exp never writes; zero them
                        nc.gpsimd.memset(ex2[:, off + d0: off + d], 0.0)
                    nc.scalar.activation(ex2[:, off + d: off + 1024],
                                         sps[:, d:1024], AF.Exp,
                                         scale=0.125, bias=bexp_sb[:, 0:1])
                    if diag:
                        # strip masking on both halves: [d:d+128) is the
                        # partially-masked boundary; the second tile's
                        # B-half cols [d0:d) stream in the DoubleRow window
                        # fully masked -> memset zero
                        nc.vector.tensor_mul(
                            ex2[:, off + d: off + d + 128],
                            ex2[:, off + d: off + d + 128],
                            mask8[:])
                        nc.vector.tensor_mul(
                            ex2[:, off + 512 + d: off + 512 + d + 128],
                            ex2[:, off + 512 + d: off + 512 + d + 128],
                            mask8[:])
                        if d > d0:
                            nc.gpsimd.memset(
                                ex2[:, off + 512 + d0: off + 512 + d], 0.0)
                attn_mm(0, m)
                if m >= 1:
                    attn_mm(1, m - 1)
                yield 0.9
            attn_mm(1, npair - 1)
            attn_drain(ic, p, accA, accB, tail=(ic == 2 and p == 3))

        # ---- scheduler: attention steps with dependency-forced + credit-
        # ---- balanced filler chunks (V/QK projections, output projection)
        from collections import deque

        emitted = set()

        def emit(item):
            if item in emitted:
                return 0.0
            emitted.add(item)
            kind = item[0]
            if kind == "x8":
                x8_chunk(item[1])
                return 0.3
            if kind == "wv8":
                wv8_cast()
                return 0.3
            if kind == "v":
                if item[1] >= 4:
                    emit(("wv8",))
                    emit(("x8", item[1] // 4))
                v_chunk(item[1])
                return 1.7
            if kind == "qk":
                qk_chunk(item[1], item[2], item[3])
                return 1.7
            if kind == "op":
                oproj_half(item[1], item[2], item[3], tail=(item[1] == 2))
                return 0.9
            raise AssertionError(item)

        groups = []
        for wave in range(1, NPAIR + SC):
            for ic in range(SC):
                p = wave - 1 - ic
                if 0 <= p < NPAIR:
                    groups.append((ic, p))
        # run (3,3) before (2,3): its drain then releases the ic=3 output
        # projection as filler work for the true final group
        i33, i23 = groups.index((3, 3)), groups.index((2, 3))
        groups[i23], groups[i33] = groups[i33], groups[i23]

        # soft fill order: chunks just ahead of the wave that needs them;
        # fp8 input casts go first so they run in the early idle DVE window
        fillq = deque()
        for st in range(4):
            fillq.append(("v", st))
        fillq.append(("wv8",))
        fillq.append(("x8", 1))
        added_v = 4
        for gi, (ic, p) in enumerate(groups):
            need_v = min(ST, 4 * ic + 8)   # stay a chunk ahead
            while added_v < need_v:
                if added_v % 4 == 0:
                    fillq.append(("x8", added_v // 4))
                fillq.append(("v", added_v))
                added_v += 1
            fillq.append(("qk", "q", p, ic))
            fillq.append(("qk", "k", p, ic))
        while added_v < ST:
            fillq.append(("v", added_v))
            added_v += 1

        drained = {ic: 0 for ic in range(SC)}
        credits = 0.0

        def force(item):
            nonlocal credits
            credits -= emit(item)

        # PE clock warmup: dummy matmuls run during the input-DMA wait so
        # the HAM clock gate releases (1.2 -> 2.4 GHz) before real work
        wps = ps_scores.tile([P, 1024], f32, tag="scores", name="warmps")
        for i in range(24):
            nc.tensor.matmul(wps[:, 0:512], warm_sb[:, 0:P], warm_sb[:],
                             start=(i == 0), stop=(i == 23))

        def force_deps(ic, p):
            for st in range(min(ST, 4 * ic + 4)):
                force(("v", st))
            force(("qk", "q", p, ic))
            for sc in range(ic + 1):
                force(("qk", "k", p, sc))

        for gi, (ic, p) in enumerate(groups):
            force_deps(ic, p)
            if gi + 1 < len(groups):
                # prefetch the next group's inputs now: their GpSimd bias
                # adds must clear that queue before its scores need them
                force_deps(*groups[gi + 1])
            stepper = (attn_steps_bf16(ic, p) if ic == 0
                       else attn_steps_fp8(ic, p, hb=(gi >= 10)))
            for credit in stepper:
                credits += credit
                while credits > 0 and fillq:
                    item = fillq.popleft()
                    credits -= emit(item)
            drained[ic] += 1
            if drained[ic] == NPAIR:
                for st in range(4 * ic, 4 * ic + 4):
                    for mc in range(2):
                        fillq.append(("op", ic, st, mc))
        hbp = ps_small.tile([P, 512], f32, tag="work", name="hbtail")
        for i in range(14):
            nc.tensor.matmul(hbp[:], warm_sb[:, 0:P], warm_sb[:],
                             start=(i == 0), stop=(i == 13))
        while fillq:
            emit(fillq.popleft())

    nc.compile()
    return nc


def _get_nc():
    if "nc" not in _CACHE:
        _CACHE["nc"] = _build()
    return _CACHE["nc"]


def _ensure_ntff_hook():
    # bass_utils' trace path imports antenv.axon_hooks, which this image's
    # antenv package lacks; recreate the tiny get/set module and install the
    # ctypes NTFF hook the boot shim would have registered.
    import sys
    import types
    try:
        import antenv.axon_hooks  # noqa: F401
        return
    except ImportError:
        pass
    try:
        import antenv
        mod = types.ModuleType("antenv.axon_hooks")
        mod._hook = None
        def set_axon_ntff_profile_hook(h, _m=mod):
            _m._hook = h
        def get_axon_ntff_profile_hook(_m=mod):
            return _m._hook
        mod.set_axon_ntff_profile_hook = set_axon_ntff_profile_hook
        mod.get_axon_ntff_profile_hook = get_axon_ntff_profile_hook
        sys.modules["antenv.axon_hooks"] = mod
        antenv.axon_hooks = mod
        try:
            from trn_agent_boot.trn_boot import _ntff_profile_via_ctypes
            h = _ntff_profile_via_ctypes("/opt/axon/libaxon_pjrt.so")
            if h is not None:
                set_axon_ntff_profile_hook(h)
        except Exception:
            pass
    except Exception:
        pass


def kernel(x, mask, Wq, bq, Wk, bk, Wv, bv, Wo, bo, **_unused):
    global LAST_EXEC_NS
    import ml_dtypes
    from concourse.bass_utils import run_bass_kernel_spmd

    _ensure_ntff_hook()

    bff = ml_dtypes.bfloat16
    x = np.asarray(x, dtype=np.float32)
    Wq = np.asarray(Wq, dtype=np.float32)
    Wk = np.asarray(Wk, dtype=np.float32)
    Wv = np.asarray(Wv, dtype=np.float32)
    Wo = np.asarray(Wo, dtype=np.float32)
    bq = np.asarray(bq, dtype=np.float32)
    bk = np.asarray(bk, dtype=np.float32)
    bv = np.asarray(bv, dtype=np.float32)
    bo = np.asarray(bo, dtype=np.float32)

    xts = [np.ascontiguousarray(x[b].T.astype(bff)) for b in range(B)]

    nc = _get_nc()
    in_maps = []
    for c in range(N_CORES):
        b, g = c // 2, c % 2
        r = slice(g * DH, (g + 1) * DH)
        in_maps.append({
            "xt": xts[b],
            "wq": np.ascontiguousarray(Wq[r].T.astype(bff)),
            "wk": np.ascontiguousarray(Wk[r].T.astype(bff)),
            "wv": np.ascontiguousarray(Wv[r].T.astype(bff)),
            "wo": np.ascontiguousarray(Wo[:, r].T.astype(bff)),
            "bq": np.ascontiguousarray(bq[r]),
            "bk": np.ascontiguousarray(bk[r]),
            "bv": np.ascontiguousarray(bv[r]),
        })

    res = run_bass_kernel_spmd(nc, in_maps, list(range(N_CORES)),
                               trace=bool(os.environ.get("BASS_TRACE")))
    LAST_EXEC_NS = res.exec_time_ns

    out = np.zeros((B, S, D), dtype=np.float32)
    for c in range(N_CORES):
        out[c // 2] += res.results[c]["y"].astype(np.float32)
    out += bo[None, None, :]
    return out


# revision 63
# speedup vs baseline: 1.0120x; 1.0120x over previous
"""Multi-head attention (B=4, S=2048, D=1024, H=16, causal) on 8 TRN2 NeuronCores.

Sharding: core c -> batch b = c // 2, head-group g = c % 2 (8 heads, 512 dims).
Each core computes its heads' projections + full SxS causal attention + its
partial output projection; the host sums the two head-group partials per batch
and adds the output bias.

Host-side prep: x and all weights are cast to bf16 and pre-transposed into the
layouts the PE consumes directly (x^T [D,S], W^T [D,DH], Wo^T [DH,D]), so the
kernel does zero on-device transposes and the input DMA bytes are halved.

Per-core pipeline (matmuls bf16 with fp32 PSUM accumulate; attn@V in fp8):
  - Q^T/K^T per head-pair slabs [128, S]; V kept natural with a ones column
    per head (ones-augmented V makes attn@V also produce sumexp rows)
  - scores^T tiles [128 j, 512 i] via two K=64 row-packed matmuls (both heads
    of a pair concurrently in the PE array); diagonal tiles only computed
    from their first in-range column
  - exp on ScalarE straight out of PSUM (no max subtraction: |s/8| <= 9.47
    verified empirically); fp8 path biases by -6*ln2 so exp fits fp8e4's
    +-240 range (the softmax normalization cancels the factor)
  - attn@V for i-chunks 1..3 in fp8e4 with MatmulPerfMode.DoubleRow: two
    128-row j-tiles contracted per pass (2x PE throughput). i-chunk 0 (rows
    0-511) stays bf16: its rows average few keys, so fp8 noise would not be
    sqrt(n)-attenuated there.
  - softmax normalization: 1/sumexp via one reciprocal_approx_fast, then
    gpsimd partition_broadcast, then one DVE multiply per head
  - a credit-based scheduler interleaves V/QK-projection and output-
    projection chunks between attention pair-steps, so ScalarE's exp stream
    and the PE matmul stream stay dense simultaneously
  - y emitted bf16 (host accumulates the two partials per batch in fp32)
"""

import os
import numpy as np

B, S, D = 4, 2048, 1024
H, DK = 16, 64
N_CORES = 8
DH = 512          # head dims per core (8 heads x 64)
P = 128           # partitions
KT = D // P       # 8 k-slabs
NPAIR = 4         # head pairs per core
SC = S // 512     # 4 s-chunks of 512
ST = S // P       # 16 s-tiles of 128
HW = DK + 2       # 66: per-head row width in fp8 V storage (64 V + 1 ones + 1 pad)
VWP = 8 * HW      # 528: fp8 v storage row width per s-tile (16-aligned for DoubleRow)
VW16 = 8 * (DK + 1)  # 520: bf16 v storage row width (s-tiles 0..3 only)
B_EXP = -6 * 0.6931471805599453  # exp bias: keeps exp(s/8)*2^-6 <= ~202 < 240 (fp8e4 max)

_CACHE = {}
LAST_EXEC_NS = None


def _build():
    from contextlib import ExitStack

    import concourse.bass as bass
    import concourse.tile as tile
    from concourse import bacc, mybir

    f32 = mybir.dt.float32
    bf16 = mybir.dt.bfloat16
    fp8 = mybir.dt.float8e4
    AF = mybir.ActivationFunctionType
    OP = mybir.AluOpType
    DR = mybir.MatmulPerfMode.DoubleRow

    nc = bacc.Bacc("TRN2", target_bir_lowering=False, debug=False,
                   num_devices=N_CORES)

    # host-pretransposed bf16 inputs
    xt = nc.dram_tensor("xt", [D, S], bf16, kind="ExternalInput").ap()
    wq = nc.dram_tensor("wq", [D, DH], bf16, kind="ExternalInput").ap()
    wk = nc.dram_tensor("wk", [D, DH], bf16, kind="ExternalInput").ap()
    wv = nc.dram_tensor("wv", [D, DH], bf16, kind="ExternalInput").ap()
    wo = nc.dram_tensor("wo", [DH, D], bf16, kind="ExternalInput").ap()
    bq = nc.dram_tensor("bq", [DH], f32, kind="ExternalInput").ap()
    bk = nc.dram_tensor("bk", [DH], f32, kind="ExternalInput").ap()
    bv = nc.dram_tensor("bv", [DH], f32, kind="ExternalInput").ap()
    y = nc.dram_tensor("y", [S, D], bf16, kind="ExternalOutput").ap()

    with tile.TileContext(nc) as tc, ExitStack() as ctx:
        persist = ctx.enter_context(tc.tile_pool(name="persist", bufs=1))

        # persistent SBUF tensors
        xT = persist.tile([P, KT * S], bf16, tag="xT")            # k-slab k at cols [k*S, (k+1)*S)
        x8 = persist.tile([P, KT * S], fp8, tag="x8")             # fp8 copy for V projection
        wqT = persist.tile([P, KT * DH], bf16, tag="wqT")         # [128k, 512dq] per slab
        wkT = persist.tile([P, KT * DH], bf16, tag="wkT")
        wvT = persist.tile([P, KT * DH], bf16, tag="wvT")
        wv8 = persist.tile([P, KT * DH], fp8, tag="wv8")
        woT = persist.tile([P, NPAIR * D], bf16, tag="woT")       # d-slab dt at cols [dt*D, ...)
        qT = persist.tile([P, NPAIR * S], bf16, tag="qT")         # pair p at cols [p*S, ...)
        kTt = persist.tile([P, NPAIR * S], bf16, tag="kTt")
        vS = persist.tile([P, ST * VWP], fp8, tag="vS")           # s-tile jt at cols [jt*VWP, ...)
        vS16 = persist.tile([P, 4 * VW16], bf16, tag="vS16")      # bf16 V for s-tiles 0..3
        ctxT = persist.tile([P, NPAIR * S], bf16, tag="ctxT")
        # the causal boundary strip (128x128 lower-triangle; same for every
        # diagonal offset)
        mask2 = persist.tile([P, 128], bf16, tag="mask2")
        mask8 = persist.tile([P, 128], fp8, tag="mask8")
        bq_sb = persist.tile([P, NPAIR], f32, tag="bq_sb")
        bk_sb = persist.tile([P, NPAIR], f32, tag="bk_sb")
        bv_sb = persist.tile([1, DH], f32, tag="bv_sb")
        bv_bc = persist.tile([P, DH], f32, tag="bv_bc")
        ones_f = persist.tile([P, 128], f32, tag="ones_f")
        bexp_sb = persist.tile([P, 1], f32, tag="bexp_sb")

        # ---- input DMAs spread across the issue queues ----
        # wv and x-sc0 gate the first PE work: split their descriptor
        # issues across queues so both arrive ~8us in
        def _dma_x(eng, sc, k0=0, k1=KT):
            for k in range(k0, k1):
                eng.dma_start(
                    out=xT[:, k * S + sc * 512: k * S + (sc + 1) * 512],
                    in_=xt[k * P:(k + 1) * P, sc * 512:(sc + 1) * 512])
        for k in range(KT):
            nc.scalar.dma_start(out=wvT[:, k * DH:(k + 1) * DH],
                                in_=wv[k * P:(k + 1) * P, :])
        _dma_x(nc.sync, 0, 0, 4)          # sc0 k4..7 issued by gpsimd below
        # wq/wk next on sync: the first qk chunks gate the exp ramp-up
        for k in range(KT):
            nc.sync.dma_start(out=wqT[:, k * DH:(k + 1) * DH],
                              in_=wq[k * P:(k + 1) * P, :])
        for k in range(KT):
            nc.sync.dma_start(out=wkT[:, k * DH:(k + 1) * DH],
                              in_=wk[k * P:(k + 1) * P, :])
        _dma_x(nc.scalar, 1)
        _dma_x(nc.sync, 2)
        _dma_x(nc.sync, 3)
        # gpsimd: warmup scratch first, then biases + V-ones init + bv
        # broadcast (v_proj needs them), then wq/wk/wo, then the mask/
        # constant prep (needed later)
        warm_sb = persist.tile([P, 512], bf16, tag="warm_sb")
        nc.gpsimd.memset(warm_sb[:], 0.5)
        nc.gpsimd.dma_start(out=bq_sb[:], in_=bq[:].rearrange("(c p) -> p c", p=P))
        nc.gpsimd.dma_start(out=bk_sb[:], in_=bk[:].rearrange("(c p) -> p c", p=P))
        nc.gpsimd.dma_start(out=bv_sb[0:1, :], in_=bv[:])
        # only the ones columns need initializing; V columns are written by
        # the projection
        nc.gpsimd.memset(
            vS[:].rearrange("p (t h c) -> p t h c", h=8, c=HW)[:, :, :, DK:DK + 1], 1.0)
        nc.gpsimd.memset(
            vS16[:].rearrange("p (t h c) -> p t h c", h=8, c=DK + 1)[:, :, :, DK:DK + 1], 1.0)
        nc.gpsimd.partition_broadcast(bv_bc[:], bv_sb[0:1, :])
        _dma_x(nc.gpsimd, 0, 4, KT)
        for dt in range(NPAIR):
            nc.gpsimd.dma_start(out=woT[:, dt * D:(dt + 1) * D],
                                in_=wo[dt * P:(dt + 1) * P, :])
        nc.gpsimd.memset(ones_f[:], 1.0)
        nc.gpsimd.memset(bexp_sb[:], B_EXP)
        for mk in (mask2, mask8):
            # keep iff strip-local column >= channel (lower triangle)
            nc.gpsimd.affine_select(
                out=mk[:],
                in_=ones_f[:],
                pattern=[[1, 128]],
                compare_op=OP.is_ge,
                fill=0.0,
                base=0,
                channel_multiplier=-1,
            )

        ps_small = ctx.enter_context(tc.tile_pool(name="ps_small", bufs=2, space="PSUM"))
        ps_scores = ctx.enter_context(tc.tile_pool(name="ps_scores", bufs=2, space="PSUM"))
        ps_acc = ctx.enter_context(tc.tile_pool(name="ps_acc", bufs=2, space="PSUM"))
        sb_vtmp = ctx.enter_context(tc.tile_pool(name="sb_vtmp", bufs=2))
        sb_exp8 = ctx.enter_context(tc.tile_pool(name="sb_exp8", bufs=7))
        sb_exp16 = ctx.enter_context(tc.tile_pool(name="sb_exp16", bufs=6))
        sb_y = ctx.enter_context(tc.tile_pool(name="sb_y", bufs=4))
        sb_rab = ctx.enter_context(tc.tile_pool(name="sb_rab", bufs=2))
        sb_rsb = ctx.enter_context(tc.tile_pool(name="sb_rsb", bufs=3))

        # ---- work-chunk emitters (filler units for the scheduler) ----
        def x8_chunk(sc):
            # fp8 copy of x for the DoubleRow V projection
            for k in range(KT):
                nc.vector.tensor_copy(
                    x8[:, k * S + sc * 512: k * S + (sc + 1) * 512],
                    xT[:, k * S + sc * 512: k * S + (sc + 1) * 512])

        def wv8_cast():
            # split so no single long cast blocks the DVE FIFO
            for k in range(0, KT, 2):
                nc.vector.tensor_copy(wv8[:, k * DH:(k + 2) * DH],
                                      wvT[:, k * DH:(k + 2) * DH])

        def v_chunk(st):
            vp = ps_small.tile([P, 512], f32, tag="work", name=f"vps{st}")
            if st < 4:
                for k in range(KT):
                    nc.tensor.matmul(
                        vp[:],
                        xT[:, k * S + st * P: k * S + (st + 1) * P],
                        wvT[:, k * DH:(k + 1) * DH],
                        start=(k == 0), stop=(k == KT - 1))
            else:
                # fp8 DoubleRow: two k-slabs contracted per pass
                for k2 in range(KT // 2):
                    lhs = x8[:, 2 * k2 * S: (2 * k2 + 2) * S].rearrange(
                        "p (two s) -> p two s", two=2)[:, :, st * P:(st + 1) * P]
                    rhs = wv8[:, 2 * k2 * DH: (2 * k2 + 2) * DH].rearrange(
                        "p (two d) -> p two d", two=2)
                    nc.tensor.matmul(
                        vp[:], lhs, rhs, perf_mode=DR,
                        start=(k2 == 0), stop=(k2 == KT // 2 - 1))
            vdst8 = vS[:, st * VWP:(st + 1) * VWP].rearrange(
                "p (h c) -> p h c", c=HW)[:, :, 0:DK]
            if st < 4:
                vt = sb_vtmp.tile([P, 512], f32, tag="vtmp", name=f"vt{st}")
                nc.vector.tensor_tensor(vt[:], vp[:], bv_bc[:], OP.add)
                nc.vector.tensor_copy(vdst8, vt[:].rearrange("p (h c) -> p h c", c=DK))
                vdst16 = vS16[:, st * VW16:(st + 1) * VW16].rearrange(
                    "p (h c) -> p h c", c=DK + 1)[:, :, 0:DK]
                nc.scalar.copy(vdst16, vt[:].rearrange("p (h c) -> p h c", c=DK))
            else:
                # fused bias-add + fp8 cast straight out of PSUM
                nc.vector.tensor_tensor(
                    vdst8,
                    vp[:].rearrange("p (h c) -> p h c", c=DK),
                    bv_bc[:].rearrange("p (h c) -> p h c", c=DK),
                    OP.add)

        def qk_chunk(name, p, sc):
            wT, bias_sb, out_sb = ((wqT, bq_sb, qT) if name == "q"
                                   else (wkT, bk_sb, kTt))
            pw = ps_small.tile([P, 512], f32, tag="work", name=f"{name}ps{p}_{sc}")
            for k in range(KT):
                nc.tensor.matmul(
                    pw[:],
                    wT[:, k * DH + p * P: k * DH + (p + 1) * P],
                    xT[:, k * S + sc * 512: k * S + (sc + 1) * 512],
                    start=(k == 0), stop=(k == KT - 1))
            nc.vector.tensor_scalar_add(
                out_sb[:, p * S + sc * 512: p * S + (sc + 1) * 512],
                pw[:], bias_sb[:, p:p + 1])

        def oproj_half(ic, st, mc, tail=False):
            yp = ps_small.tile([P, 512], f32, tag="work", name=f"yp{st}_{mc}")
            for dt in range(NPAIR):
                nc.tensor.matmul(
                    yp[:],
                    ctxT[:, dt * S + st * P: dt * S + (st + 1) * P],
                    woT[:, dt * D + mc * 512: dt * D + (mc + 1) * 512],
                    start=(dt == 0), stop=(dt == NPAIR - 1))
            yt = sb_y.tile([P, 512], bf16, tag="yout", name=f"yt{st}_{mc}")
            nc.vector.tensor_copy(yt[:], yp[:])
            eng = nc.gpsimd if tail else nc.sync
            eng.dma_start(
                out=y[st * P:(st + 1) * P, mc * 512:(mc + 1) * 512], in_=yt[:])

        # ---- attention group step generators ----
        def attn_drain(ic, p, accA, accB, tail=False):
            cslice = slice(p * S + ic * 512, p * S + (ic + 1) * 512)
            # sumexp rows first: the gpsimd broadcast chain is the critical
            # path, the ctx copies slot in behind it on the DVE FIFO
            sraw = sb_rab.tile([1, 1024], f32, tag="sraw", name=f"sr{ic}{p}")
            nc.vector.tensor_copy(sraw[0:1, 0:512], accA[DK:DK + 1, :])
            nc.vector.tensor_copy(sraw[0:1, 512:1024], accB[DK:DK + 1, :])
            Rsr = sb_rsb.tile([P, 1024], f32, tag="rsb", name=f"rr{ic}{p}")
            nc.gpsimd.partition_broadcast(Rsr[:], sraw[0:1, :])
            if tail:
                # ScalarE has finished its exps by the final drain
                nc.scalar.copy(ctxT[0:DK, cslice], accA[0:DK, :])
                nc.scalar.copy(ctxT[DK:P, cslice], accB[0:DK, :])
            else:
                nc.vector.tensor_copy(ctxT[0:DK, cslice], accA[0:DK, :])
                nc.vector.tensor_copy(ctxT[DK:P, cslice], accB[0:DK, :])
            # reciprocal across all 128 lanes (a [1,N] reciprocal would run
            # on a single DVE lane)
            Rs = sb_rsb.tile([P, 1024], f32, tag="rsb", name=f"rs{ic}{p}")
            nc.vector.reciprocal_approx_fast(Rs[:], Rsr[:])
            for hl in (0, 1):
                csl = ctxT[hl * DK:(hl + 1) * DK, cslice]
                nc.vector.tensor_mul(
                    csl, csl, Rs[hl * DK:(hl + 1) * DK, hl * 512:(hl + 1) * 512])

        def attn_steps_bf16(ic, p):
            # per-tile bf16 path (ic=0 only: rows 0-511)
            accA = ps_acc.tile([DK + 1, 512], f32, tag="acc", name=f"accA{ic}_{p}")
            accB = ps_acc.tile([DK + 1, 512], f32, tag="acc", name=f"accB{ic}_{p}")
            njt = 4 * ic + 4
            exs = {}
            dof = {}

            def attn_mm(hl, jt):
                acc = accA if hl == 0 else accB
                d = dof[jt]
                hv = 2 * p + hl
                nc.tensor.matmul(
                    acc[:, d:512],
                    vS16[:, jt * VW16 + hv * (DK + 1): jt * VW16 + (hv + 1) * (DK + 1)],
                    exs[jt][:, hl * 512 + d:(hl + 1) * 512],
                    start=(jt == 0), stop=(jt == njt - 1))

            for jt in range(njt):
                sps = ps_scores.tile([P, 1024], f32, tag="scores", name=f"sps{ic}{p}{jt}")
                diag = jt >= 4 * ic
                d = (jt - 4 * ic) * P if diag else 0
                dof[jt] = d
                nc.tensor.matmul(
                    sps[:, d:512],
                    kTt[0:DK, p * S + jt * P: p * S + (jt + 1) * P],
                    qT[0:DK, p * S + ic * 512 + d: p * S + (ic + 1) * 512],
                    start=True, stop=True)
                nc.tensor.matmul(
                    sps[:, 512 + d:1024],
                    kTt[DK:P, p * S + jt * P: p * S + (jt + 1) * P],
                    qT[DK:P, p * S + ic * 512 + d: p * S + (ic + 1) * 512],
                    start=True, stop=True)
                ex = sb_exp16.tile([P, 1024], bf16, tag="exp16", name=f"ex{ic}{p}{jt}")
                exs[jt] = ex
                nc.scalar.activation(ex[:, d:1024], sps[:, d:1024], AF.Exp, scale=0.125)
                if diag:
                    # only the 128-wide boundary strip is partially masked;
                    # columns beyond d+128 are fully in-range
                    for hb in (0, 512):
                        nc.vector.tensor_mul(
                            ex[:, hb + d: hb + d + 128],
                            ex[:, hb + d: hb + d + 128],
                            mask2[:])
                attn_mm(0, jt)
                if jt >= 2:
                    attn_mm(1, jt - 2)
                yield 0.15
            for jt in range(max(0, njt - 2), njt):
                attn_mm(1, jt)
            attn_drain(ic, p, accA, accB)

        def attn_steps_fp8(ic, p, hb=False):
            # fp8 DoubleRow path: j-tiles consumed in pairs, 2x PE throughput
            accA = ps_acc.tile([DK + 1, 512], f32, tag="acc", name=f"accA{ic}_{p}")
            accB = ps_acc.tile([DK + 1, 512], f32, tag="acc", name=f"accB{ic}_{p}")
            npair = 2 * ic + 2
            exs = {}
            wss = {}

            def attn_mm(hl, m):
                acc = accA if hl == 0 else accB
                ws = wss[m]
                hv = 2 * p + hl
                lhs = vS[:, 2 * m * VWP: (2 * m + 2) * VWP].rearrange(
                    "p (two w) -> p two w", two=2)[:, :, hv * HW: hv * HW + DK + 1]
                rhs = exs[m].rearrange(
                    "p (two w) -> p two w", two=2)[:, :, hl * 512 + ws: (hl + 1) * 512]
                nc.tensor.matmul(
                    acc[:, ws:512], lhs, rhs, perf_mode=DR,
                    start=(m == 0), stop=(m == npair - 1))

            for m in range(npair):
                jts = (2 * m, 2 * m + 1)
                ex2 = sb_exp8.tile([P, 2048], fp8, tag="exp8", name=f"e2{ic}{p}{m}")
                exs[m] = ex2
                d0 = (jts[0] - 4 * ic) * P if jts[0] >= 4 * ic else 0
                wss[m] = d0
                for q01, jt in enumerate(jts):
                    diag = jt >= 4 * ic
                    d = (jt - 4 * ic) * P if diag else 0
                    sps = ps_scores.tile([P, 1024], f32, tag="scores",
                                         name=f"sps{ic}{p}{jt}")
                    if hb and q01 == 0:
                        # dummy matmul: overwritten by the real start=True
                        # below; exists only to hold the PE clock at 2.4GHz
                        nc.tensor.matmul(sps[:, 0:512], warm_sb[:, 0:P],
                                         warm_sb[:], start=True, stop=True)
                    nc.tensor.matmul(
                        sps[:, d:512],
                        kTt[0:DK, p * S + jt * P: p * S + (jt + 1) * P],
                        qT[0:DK, p * S + ic * 512 + d: p * S + (ic + 1) * 512],
                        start=True, stop=True)
                    nc.tensor.matmul(
                        sps[:, 512 + d:1024],
                        kTt[DK:P, p * S + jt * P: p * S + (jt + 1) * P],
                        qT[DK:P, p * S + ic * 512 + d: p * S + (ic + 1) * 512],
                        start=True, stop=True)
                    off = q01 * 1024
                    if diag and d > d0:
                        # window [d0:512] streams this tile's A-side cols
                        # [d0:d) which exp never writes; zero them
                        nc.gpsimd.memset(ex2[:, off + d0: off + d], 0.0)
                    nc.scalar.activation(ex2[:, off + d: off + 1024],
                                         sps[:, d:1024], AF.Exp,
                                         scale=0.125, bias=bexp_sb[:, 0:1])
                    if diag:
                        # strip masking on both halves: [d:d+128) is the
                        # partially-masked boundary; the second tile's
                        # B-half cols [d0:d) stream in the DoubleRow window
                        # fully masked -> memset zero
                        nc.vector.tensor_mul(
                            ex2[:, off + d: off + d + 128],
                            ex2[:, off + d: off + d + 128],
                            mask8[:])
                        nc.vector.tensor_mul(
                            ex2[:, off + 512 + d: off + 512 + d + 128],
                            ex2[:, off + 512 + d: off + 512 + d + 128],
                            mask8[:])
                        if d > d0:
                            nc.gpsimd.memset(
                                ex2[:, off + 512 + d0: off + 512 + d], 0.0)
                attn_mm(0, m)
                if m >= 1:
                    attn_mm(1, m - 1)
                yield 0.9
            attn_mm(1, npair - 1)
            attn_drain(ic, p, accA, accB, tail=(ic == 2 and p == 3))

        # ---- scheduler: attention steps with dependency-forced + credit-
        # ---- balanced filler chunks (V/QK projections, output projection)
        from collections import deque

        emitted = set()

        def emit(item):
            if item in emitted:
                return 0.0
            emitted.add(item)
            kind = item[0]
            if kind == "x8":
                x8_chunk(item[1])
                return 0.3
            if kind == "wv8":
                wv8_cast()
                return 0.3
            if kind == "v":
                if item[1] >= 4:
                    emit(("wv8",))
                    emit(("x8", item[1] // 4))
                v_chunk(item[1])
                return 1.7
            if kind == "qk":
                qk_chunk(item[1], item[2], item[3])
                return 1.7
            if kind == "op":
                oproj_half(item[1], item[2], item[3], tail=(item[1] == 2))
                return 0.9
            raise AssertionError(item)

        groups = []
        for wave in range(1, NPAIR + SC):
            for ic in range(SC):
                p = wave - 1 - ic
                if 0 <= p < NPAIR:
                    groups.append((ic, p))
        # run (3,3) before (2,3): its drain then releases the ic=3 output
        # projection as filler work for the true final group
        i33, i23 = groups.index((3, 3)), groups.index((2, 3))
        groups[i23], groups[i33] = groups[i33], groups[i23]

        # soft fill order: chunks just ahead of the wave that needs them;
        # fp8 input casts go first so they run in the early idle DVE window
        fillq = deque()
        for st in range(4):
            fillq.append(("v", st))
        fillq.append(("wv8",))
        fillq.append(("x8", 1))
        added_v = 4
        for gi, (ic, p) in enumerate(groups):
            need_v = min(ST, 4 * ic + 8)   # stay a chunk ahead
            while added_v < need_v:
                if added_v % 4 == 0:
                    fillq.append(("x8", added_v // 4))
                fillq.append(("v", added_v))
                added_v += 1
            fillq.append(("qk", "q", p, ic))
            fillq.append(("qk", "k", p, ic))
        while added_v < ST:
            fillq.append(("v", added_v))
            added_v += 1

        drained = {ic: 0 for ic in range(SC)}
        credits = 0.0

        def force(item):
            nonlocal credits
            credits -= emit(item)

        # PE clock warmup: dummy matmuls run during the input-DMA wait so
        # the HAM clock gate releases (1.2 -> 2.4 GHz) before real work
        wps = ps_scores.tile([P, 1024], f32, tag="scores", name="warmps")
        for i in range(18):
            nc.tensor.matmul(wps[:, 0:512], warm_sb[:, 0:P], warm_sb[:],
                             start=(i == 0), stop=(i == 17))

        def force_deps(ic, p):
            for st in range(min(ST, 4 * ic + 4)):
                force(("v", st))
            force(("qk", "q", p, ic))
            for sc in range(ic + 1):
                force(("qk", "k", p, sc))

        for gi, (ic, p) in enumerate(groups):
            force_deps(ic, p)
            if gi + 1 < len(groups):
                # prefetch the next group's inputs now: their GpSimd bias
                # adds must clear that queue before its scores need them
                force_deps(*groups[gi + 1])
            stepper = (attn_steps_bf16(ic, p) if ic == 0
                       else attn_steps_fp8(ic, p, hb=(gi >= 10)))
            for credit in stepper:
                credits += credit
                while credits > 0 and fillq:
                    item = fillq.popleft()
                    credits -= emit(item)
            drained[ic] += 1
            if drained[ic] == NPAIR:
                for st in range(4 * ic, 4 * ic + 4):
                    for mc in range(2):
                        fillq.append(("op", ic, st, mc))
        hbp = ps_small.tile([P, 512], f32, tag="work", name="hbtail")
        for i in range(14):
            nc.tensor.matmul(hbp[:], warm_sb[:, 0:P], warm_sb[:],
                             start=(i == 0), stop=(i == 13))
        while fillq:
            emit(fillq.popleft())

    nc.compile()
    return nc


def _get_nc():
    if "nc" not in _CACHE:
        _CACHE["nc"] = _build()
    return _CACHE["nc"]


def _ensure_ntff_hook():
    # bass_utils' trace path imports antenv.axon_hooks, which this image's
    # antenv package lacks; recreate the tiny get/set module and install the
    # ctypes NTFF hook the boot shim would have registered.
    import sys
    import types
    try:
        import antenv.axon_hooks  # noqa: F401
        return
    except ImportError:
        pass
    try:
        import antenv
        mod = types.ModuleType("antenv.axon_hooks")
        mod._hook = None
        def set_axon_ntff_profile_hook(h, _m=mod):
            _m._hook = h
        def get_axon_ntff_profile_hook(_m=mod):
            return _m._hook
        mod.set_axon_ntff_profile_hook = set_axon_ntff_profile_hook
        mod.get_axon_ntff_profile_hook = get_axon_ntff_profile_hook
        sys.modules["antenv.axon_hooks"] = mod
        antenv.axon_hooks = mod
        try:
            from trn_agent_boot.trn_boot import _ntff_profile_via_ctypes
            h = _ntff_profile_via_ctypes("/opt/axon/libaxon_pjrt.so")
            if h is not None:
                set_axon_ntff_profile_hook(h)
        except Exception:
            pass
    except Exception:
        pass


def kernel(x, mask, Wq, bq, Wk, bk, Wv, bv, Wo, bo, **_unused):
    global LAST_EXEC_NS
    import ml_dtypes
    from concourse.bass_utils import run_bass_kernel_spmd

    _ensure_ntff_hook()

    bff = ml_dtypes.bfloat16
    x = np.asarray(x, dtype=np.float32)
    Wq = np.asarray(Wq, dtype=np.float32)
    Wk = np.asarray(Wk, dtype=np.float32)
    Wv = np.asarray(Wv, dtype=np.float32)
    Wo = np.asarray(Wo, dtype=np.float32)
    bq = np.asarray(bq, dtype=np.float32)
    bk = np.asarray(bk, dtype=np.float32)
    bv = np.asarray(bv, dtype=np.float32)
    bo = np.asarray(bo, dtype=np.float32)

    xts = [np.ascontiguousarray(x[b].T.astype(bff)) for b in range(B)]

    nc = _get_nc()
    in_maps = []
    for c in range(N_CORES):
        b, g = c // 2, c % 2
        r = slice(g * DH, (g + 1) * DH)
        in_maps.append({
            "xt": xts[b],
            "wq": np.ascontiguousarray(Wq[r].T.astype(bff)),
            "wk": np.ascontiguousarray(Wk[r].T.astype(bff)),
            "wv": np.ascontiguousarray(Wv[r].T.astype(bff)),
            "wo": np.ascontiguousarray(Wo[:, r].T.astype(bff)),
            "bq": np.ascontiguousarray(bq[r]),
            "bk": np.ascontiguousarray(bk[r]),
            "bv": np.ascontiguousarray(bv[r]),
        })

    res = run_bass_kernel_spmd(nc, in_maps, list(range(N_CORES)),
                               trace=bool(os.environ.get("BASS_TRACE")))
    LAST_EXEC_NS = res.exec_time_ns

    out = np.zeros((B, S, D), dtype=np.float32)
    for c in range(N_CORES):
        out[c // 2] += res.results[c]["y"].astype(np.float32)
    out += bo[None, None, :]
    return out
